# revision 1
# baseline (speedup 1.0000x reference)
"""Trainium2 Bass kernel for ragged subword mean pooling (nn_Bert).

Problem: out[b, j] = mean(bert_embedding[b, st_j:ed_j]) if (mask & ed>st) else 0
Shapes: bert_embedding [32, 1024, 768] f32, x_bert_offset [32, 768, 2] i32,
        x_mask [32, 768] i32 -> out [32, 768, 768] f32.

Strategy (pure data parallel, 4 batch rows per core on 8 cores):
  Spans are contiguous sorted segments, so per row the pooling is
  out = A.T @ E where A[s, j] = scale_j iff st_j <= s < ed_j
  (scale_j = valid/len folds the mean and mask directly into A).
  Each position s belongs to at most ONE word, so every A tile has at
  most one nonzero per partition row. The host ships just that
  (column, value) pair per position (~32KB/core) and the device
  reconstructs each [128, win] A window in a single fused DVE op
  against a constant column-index tile J:
      A[p, j] = (J[p, j] == idx_p) * val_p
  The contraction runs on the PE in float32r (full rate; values are
  rounded to ~tf32, rel err ~1e-4). PSUM is drained by plain scalar-
  engine copies. Only (m, k) tile pairs whose word/position ranges
  intersect are computed; the active-pair hull is derived on the host
  from the actual offsets (a superset is always correct since A is 0
  outside).
"""

import sys

if "/opt/trn_rl_repo" not in sys.path:
    sys.path.insert(0, "/opt/trn_rl_repo")

import numpy as np

B, S, W, D = 32, 1024, 768, 768
NCORES = 8
RPC = B // NCORES  # rows per core
KT = S // 128  # 8 k-tiles (positions)
MT = W // 128  # 6 m-tiles (words)

_CACHE = {}


def _active_pairs(st, ed):
    """Per row-slot r: hull of active k-tiles for each m-tile, and hull of
    active m-tiles for each k-tile, unioned over cores (the SPMD program is
    shared by all 8 cores). A superset only costs time, never correctness.
    """
    kl = []
    for r in range(RPC):
        per_m = []
        for m in range(MT):
            klo, khi = KT, 0
            for c in range(NCORES):
                b = c * RPC + r
                s0 = int(st[b, m * 128 : (m + 1) * 128].min())
                s1 = int(ed[b, m * 128 : (m + 1) * 128].max())
                if s1 > s0:
                    klo = min(klo, s0 // 128)
                    khi = max(khi, (s1 + 127) // 128)
            per_m.append((klo, khi) if khi > klo else None)
        kl.append(per_m)

    mw = []
    for r in range(RPC):
        per_k = []
        for k in range(KT):
            mlo, mhi = MT, 0
            for m in range(MT):
                if kl[r][m] and kl[r][m][0] <= k < kl[r][m][1]:
                    mlo = min(mlo, m)
                    mhi = max(mhi, m + 1)
            per_k.append((mlo, mhi) if mhi > mlo else None)
        mw.append(per_k)
    return kl, mw


def build_program(pairs, repeat=1, drain="act", io="ext", stage=3, nodma=False,
                  ebufs=7, abufs=8, psbufs=3, obufs=6, avbufs=2):
    """Build the SPMD Bass program (one program, run on all 8 cores)."""
    import concourse.tile as tile
    from concourse import bacc, mybir

    kl, mw = pairs
    f32 = mybir.dt.float32
    f32r = mybir.dt.float32r
    i32 = mybir.dt.int32
    AF = mybir.ActivationFunctionType
    OP = mybir.AluOpType

    nc = bacc.Bacc(
        "TRN2", target_bir_lowering=False, debug=False, num_devices=NCORES
    )

    E_in = nc.dram_tensor("E_in", [RPC, S, D], f32r, kind="ExternalInput").ap()
    # packed per (r, k): column 2*(r*KT+k) = one-hot column index within the
    # A window (or -1), column +1 = A value (scale of the word at that
    # position, 0 if masked/empty/uncovered)
    av_in = nc.dram_tensor("av_in", [128, RPC * KT * 2], f32, kind="ExternalInput").ap()
    if io == "ext":
        out = nc.dram_tensor("out", [RPC, W, D], f32, kind="ExternalOutput").ap()
        tok = None
    else:
        out = nc.dram_tensor("out_scratch", [RPC, W, D], f32).ap()
        tok = nc.dram_tensor("tok", [128, 16], f32, kind="ExternalOutput").ap()
    outdma = not nodma

    def win(r, k):
        if mw[r][k] is None:
            return None
        mlo, mhi = mw[r][k]
        return mlo * 128, (mhi - mlo) * 128

    awidth = 128
    for r in range(RPC):
        for k in range(KT):
            if mw[r][k]:
                awidth = max(awidth, (mw[r][k][1] - mw[r][k][0]) * 128)

    any_empty_m = any(kl[r][m] is None for r in range(RPC) for m in range(MT))

    with tile.TileContext(nc) as tc:
        with (
            tc.tile_pool(name="const", bufs=1) as cpool,
            tc.tile_pool(name="E", bufs=ebufs) as epool,
            tc.tile_pool(name="bc", bufs=avbufs) as bcpool,
            tc.tile_pool(name="A", bufs=abufs) as apool,
            tc.tile_pool(name="outsb", bufs=obufs) as opool,
            tc.tile_pool(name="psum", bufs=psbufs, space="PSUM") as pspool,
        ):
            # constant column-index tile J[p, j] = j
            j_i = cpool.tile([128, awidth], i32)
            nc.gpsimd.iota(j_i[:], pattern=[[1, awidth]], base=0, channel_multiplier=0)
            j_f = cpool.tile([128, awidth], f32)
            nc.vector.tensor_copy(j_f[:], j_i[:])
            if any_empty_m or stage < 3:
                zeros = cpool.tile([128, D], f32)
                nc.vector.memset(zeros[:], 0.0)
            econst = None
            if nodma:
                econst = []
                for h in range(2):
                    tt = cpool.tile([128, 4 * D], f32r, tag=f"Ec{h}")
                    nc.vector.memset(tt[:].bitcast(f32), 0.5)
                    econst.append(tt)

            last_at = None
            for _ in range(repeat):
                if stage >= 0:
                    av = bcpool.tile([128, RPC * KT * 2], f32, tag="av")
                    nc.sync.dma_start(av[:], av_in[:, :])

                for r in range(RPC):
                    # E row in two batched DMAs of 4 k-tiles each
                    et = []
                    if nodma:
                        for k4 in range(KT):
                            et.append(econst[k4 // 4][:, (k4 % 4) * D : (k4 % 4 + 1) * D])
                    else:
                        for h in range(2):
                            t = epool.tile([128, 4 * D], f32r, tag="E")
                            src = E_in[r, h * 512 : (h + 1) * 512, :].rearrange(
                                "(k p) d -> p k d", p=128
                            )
                            nc.sync.dma_start(
                                t[:].rearrange("p (k d) -> p k d", d=D), src
                            )
                            for k4 in range(4):
                                et.append(t[:, k4 * D : (k4 + 1) * D])

                    # one-hot A windows, one fused DVE op per k-tile
                    ak = {}
                    for k in range(KT if stage >= 1 else 0):
                        w = win(r, k)
                        if w is None:
                            continue
                        j0, wd = w
                        c = (r * KT + k) * 2
                        at = apool.tile([128, awidth], f32r, tag="A")
                        nc.vector.tensor_scalar(
                            at[:, :wd],
                            j_f[:, :wd],
                            av[:, c : c + 1],
                            av[:, c + 1 : c + 2],
                            OP.is_equal,
                            OP.mult,
                        )
                        ak[k] = (at, j0)
                        last_at = at

                    for m in range(MT):
                        if kl[r][m] is None or stage < 2:
                            if outdma:
                                nc.sync.dma_start(
                                    out[r, m * 128 : (m + 1) * 128, :], zeros[:]
                                )
                            continue
                        klo, khi = kl[r][m]
                        ps = pspool.tile([128, D], f32, tag="ps")
                        for k in range(klo, khi):
                            at, j0 = ak[k]
                            lhsT = at[:, m * 128 - j0 : (m + 1) * 128 - j0]
                            first = k == klo
                            last = k == khi - 1
                            for n0 in range(0, D, 512):
                                n1 = min(n0 + 512, D)
                                nc.tensor.matmul(
                                    ps[:, n0:n1],
                                    lhsT,
                                    et[k][:, n0:n1],
                                    start=first,
                                    stop=last,
                                )
                        if stage < 3:
                            if outdma:
                                nc.sync.dma_start(
                                    out[r, m * 128 : (m + 1) * 128, :], zeros[:]
                                )
                            continue
                        osb = opool.tile([128, D], f32, tag="osb")
                        if drain == "act":
                            nc.scalar.activation(osb[:], ps[:], AF.Copy)
                        else:
                            nc.vector.tensor_copy(osb[:], ps[:])
                        if outdma:
                            nc.sync.dma_start(
                                out[r, m * 128 : (m + 1) * 128, :], osb[:]
                            )

            if tok is not None:
                if last_at is not None:
                    nc.sync.dma_start(tok[:], last_at[:, :16].bitcast(f32))
                else:
                    nc.sync.dma_start(tok[:], zeros[:, :16])

    nc.compile()
    return nc


def _prep(bert_embedding, x_bert_offset, x_mask):
    st = x_bert_offset[..., 0].astype(np.int64)
    ed = x_bert_offset[..., 1].astype(np.int64)
    length = ed - st
    valid = (x_mask > 0) & (length > 0)
    scale = np.where(
        valid, 1.0 / np.maximum(length, 1).astype(np.float64), 0.0
    ).astype(np.float32)
    st_ext = np.concatenate([st, ed[:, -1:]], axis=1)  # [B, W+1]

    # word index of each position (-1 if uncovered)
    word_of = np.full((B, S), -1, dtype=np.int64)
    s_idx = np.arange(S)
    for b in range(B):
        j = np.searchsorted(st_ext[b], s_idx, side="right") - 1
        ok = (j >= 0) & (j < W)
        word_of[b] = np.where(ok, j, -1)

    pairs = _active_pairs(st, ed)
    kl, mw = pairs

    E = np.ascontiguousarray(bert_embedding, dtype=np.float32)
    in_maps = []
    for c in range(NCORES):
        av = np.zeros((128, RPC * KT * 2), dtype=np.float32)
        for r in range(RPC):
            b = c * RPC + r
            for k in range(KT):
                if mw[r][k] is None:
                    continue
                j0 = mw[r][k][0] * 128
                col = (r * KT + k) * 2
                s = k * 128 + np.arange(128)
                wj = word_of[b, s]
                covered = wj >= 0
                # window hull guarantees covered words lie inside [j0, j0+wd)
                av[:, col] = np.where(covered, wj - j0, -1).astype(np.float32)
                av[:, col + 1] = np.where(
                    covered, scale[b, np.clip(wj, 0, W - 1)], 0.0
                )
        in_maps.append(
            {
                "E_in": E[c * RPC : (c + 1) * RPC],
                "av_in": av,
            }
        )
    return pairs, in_maps


def kernel(bert_embedding, x_bert_offset, x_mask):
    from concourse.bass_utils import run_bass_kernel_spmd

    bert_embedding = np.asarray(bert_embedding, dtype=np.float32)
    x_bert_offset = np.asarray(x_bert_offset)
    x_mask = np.asarray(x_mask)
    pairs, in_maps = _prep(bert_embedding, x_bert_offset, x_mask)
    key = repr(pairs)
    nc = _CACHE.get(key)
    if nc is None:
        nc = build_program(pairs)
        _CACHE[key] = nc
    res = run_bass_kernel_spmd(nc, in_maps, list(range(NCORES)))
    out = np.concatenate([res.results[c]["out"] for c in range(NCORES)], axis=0)
    return out.astype(np.float32)



# revision 14
# speedup vs baseline: 1.4955x; 1.4955x over previous
"""Trainium2 Bass kernel for ragged subword mean pooling (nn_Bert).

Problem: out[b, j] = mean(bert_embedding[b, st_j:ed_j]) if (mask & ed>st) else 0
Shapes: bert_embedding [32, 1024, 768] f32, x_bert_offset [32, 768, 2] i32,
        x_mask [32, 768] i32 -> out [32, 768, 768] f32.

Strategy (pure data parallel, 4 batch rows per core on 8 cores):
  Spans are contiguous sorted segments, so per row the pooling is
  out = A.T @ E where A[s, j] = scale_j iff st_j <= s < ed_j
  (scale_j = valid/len folds the mean and mask directly into A).
  Each position s belongs to at most ONE word, so every A tile has at
  most one nonzero per partition row. The host ships just that
  (column, value) pair per position (~32KB/core) and the device
  reconstructs each [128, win] A window in a single fused DVE op
  against a constant column-index tile J:
      A[p, j] = (J[p, j] == idx_p) * val_p
  The contraction runs on the PE in float32r (full rate; values are
  rounded to ~tf32, rel err ~1e-4). PSUM is drained by plain scalar-
  engine copies. Only (m, k) tile pairs whose word/position ranges
  intersect are computed; the active-pair hull is derived on the host
  from the actual offsets (a superset is always correct since A is 0
  outside).
"""

import sys

if "/opt/trn_rl_repo" not in sys.path:
    sys.path.insert(0, "/opt/trn_rl_repo")

import numpy as np

B, S, W, D = 32, 1024, 768, 768
NCORES = 8
RPC = B // NCORES  # rows per core
KT = S // 128  # 8 k-tiles (positions)
MT = W // 128  # 6 m-tiles (words)

_CACHE = {}


def _active_pairs(st, ed):
    """Per row-slot r: hull of active k-tiles for each m-tile, and hull of
    active m-tiles for each k-tile, unioned over cores (the SPMD program is
    shared by all 8 cores). A superset only costs time, never correctness.
    """
    kl = []
    for r in range(RPC):
        per_m = []
        for m in range(MT):
            klo, khi = KT, 0
            for c in range(NCORES):
                b = c * RPC + r
                s0 = int(st[b, m * 128 : (m + 1) * 128].min())
                s1 = int(ed[b, m * 128 : (m + 1) * 128].max())
                if s1 > s0:
                    klo = min(klo, s0 // 128)
                    khi = max(khi, (s1 + 127) // 128)
            per_m.append((klo, khi) if khi > klo else None)
        kl.append(per_m)

    mw = []
    for r in range(RPC):
        per_k = []
        for k in range(KT):
            mlo, mhi = MT, 0
            for m in range(MT):
                if kl[r][m] and kl[r][m][0] <= k < kl[r][m][1]:
                    mlo = min(mlo, m)
                    mhi = max(mhi, m + 1)
            per_k.append((mlo, mhi) if mhi > mlo else None)
        mw.append(per_k)
    return kl, mw


def build_program(pairs, repeat=1, drain="act", io="ext", stage=3, nodma=False,
                  ebufs=7, abufs=8, psbufs=3, obufs=6, avbufs=2):
    """Build the SPMD Bass program (one program, run on all 8 cores).

    All HBM I/O is fp16 (half the traffic of f32); PE contracts fp16 at
    full rate into f32 PSUM. Metadata (word indices <= 767, scales >=
    1/1024) is fp16-exact.
    """
    import concourse.tile as tile
    from concourse import bacc, mybir

    kl, mw = pairs
    f32 = mybir.dt.float32
    f16 = mybir.dt.float16
    i32 = mybir.dt.int32
    AF = mybir.ActivationFunctionType
    OP = mybir.AluOpType

    nc = bacc.Bacc(
        "TRN2", target_bir_lowering=False, debug=False, num_devices=NCORES
    )

    E_in = nc.dram_tensor("E_in", [RPC, S, D], f16, kind="ExternalInput").ap()
    # packed per (r, k): column 2*(r*KT+k) = one-hot column index within the
    # A window (or -1), column +1 = A value (scale of the word at that
    # position, 0 if masked/empty/uncovered)
    av_in = nc.dram_tensor("av_in", [128, RPC * KT * 2], f32, kind="ExternalInput").ap()
    if io == "ext":
        out = nc.dram_tensor("out", [RPC, W, D], f16, kind="ExternalOutput").ap()
        tok = None
    else:
        out = nc.dram_tensor("out_scratch", [RPC, W, D], f16).ap()
        tok = nc.dram_tensor("tok", [128, 16], f16, kind="ExternalOutput").ap()
    outdma = not nodma

    def win(r, k):
        if mw[r][k] is None:
            return None
        mlo, mhi = mw[r][k]
        return mlo * 128, (mhi - mlo) * 128

    awidth = 128
    for r in range(RPC):
        for k in range(KT):
            if mw[r][k]:
                awidth = max(awidth, (mw[r][k][1] - mw[r][k][0]) * 128)

    any_empty_m = any(kl[r][m] is None for r in range(RPC) for m in range(MT))

    with tile.TileContext(nc) as tc:
        with (
            tc.tile_pool(name="const", bufs=1) as cpool,
            tc.tile_pool(name="E", bufs=ebufs) as epool,
            tc.tile_pool(name="bc", bufs=avbufs) as bcpool,
            tc.tile_pool(name="A", bufs=abufs) as apool,
            tc.tile_pool(name="outsb", bufs=obufs) as opool,
            tc.tile_pool(name="psum", bufs=psbufs, space="PSUM") as pspool,
        ):
            # constant column-index tile J[p, j] = j
            j_i = cpool.tile([128, awidth], i32)
            nc.gpsimd.iota(j_i[:], pattern=[[1, awidth]], base=0, channel_multiplier=0)
            j_f = cpool.tile([128, awidth], f16)
            nc.vector.tensor_copy(j_f[:], j_i[:])
            if any_empty_m or stage < 3:
                zeros = cpool.tile([128, D], f16)
                nc.vector.memset(zeros[:], 0.0)
            econst = None
            if nodma:
                econst = []
                for h in range(2):
                    tt = cpool.tile([128, 4 * D], f16, tag=f"Ec{h}")
                    nc.vector.memset(tt[:], 0.5)
                    econst.append(tt)

            last_at = None
            for _ in range(repeat):
                if stage >= 0:
                    av = bcpool.tile([128, RPC * KT * 2], f32, tag="av")
                    nc.sync.dma_start(av[:], av_in[:, :])

                for r in range(RPC):
                    # E row in two batched DMAs of 4 k-tiles each
                    et = []
                    if nodma:
                        for k4 in range(KT):
                            et.append(econst[k4 // 4][:, (k4 % 4) * D : (k4 % 4 + 1) * D])
                    else:
                        for h in range(2):
                            t = epool.tile([128, 4 * D], f16, tag="E")
                            src = E_in[r, h * 512 : (h + 1) * 512, :].rearrange(
                                "(k p) d -> p k d", p=128
                            )
                            nc.sync.dma_start(
                                t[:].rearrange("p (k d) -> p k d", d=D), src
                            )
                            for k4 in range(4):
                                et.append(t[:, k4 * D : (k4 + 1) * D])

                    # one-hot A windows, one fused DVE op per k-tile
                    ak = {}
                    for k in range(KT if stage >= 1 else 0):
                        w = win(r, k)
                        if w is None:
                            continue
                        j0, wd = w
                        c = (r * KT + k) * 2
                        at = apool.tile([128, awidth], f16, tag="A")
                        nc.vector.tensor_scalar(
                            at[:, :wd],
                            j_f[:, :wd],
                            av[:, c : c + 1],
                            av[:, c + 1 : c + 2],
                            OP.is_equal,
                            OP.mult,
                        )
                        ak[k] = (at, j0)
                        last_at = at

                    for m in range(MT):
                        if kl[r][m] is None or stage < 2:
                            if outdma:
                                nc.sync.dma_start(
                                    out[r, m * 128 : (m + 1) * 128, :], zeros[:]
                                )
                            continue
                        klo, khi = kl[r][m]
                        ps = pspool.tile([128, D], f32, tag="ps")
                        for k in range(klo, khi):
                            at, j0 = ak[k]
                            lhsT = at[:, m * 128 - j0 : (m + 1) * 128 - j0]
                            first = k == klo
                            last = k == khi - 1
                            for n0 in range(0, D, 512):
                                n1 = min(n0 + 512, D)
                                nc.tensor.matmul(
                                    ps[:, n0:n1],
                                    lhsT,
                                    et[k][:, n0:n1],
                                    start=first,
                                    stop=last,
                                )
                        if stage < 3:
                            if outdma:
                                nc.sync.dma_start(
                                    out[r, m * 128 : (m + 1) * 128, :], zeros[:]
                                )
                            continue
                        osb = opool.tile([128, D], f16, tag="osb")
                        if drain == "act":
                            nc.scalar.activation(osb[:], ps[:], AF.Copy)
                        else:
                            nc.vector.tensor_copy(osb[:], ps[:])
                        if outdma:
                            nc.sync.dma_start(
                                out[r, m * 128 : (m + 1) * 128, :], osb[:]
                            )

            if tok is not None:
                if last_at is not None:
                    nc.sync.dma_start(tok[:], last_at[:, :16])
                else:
                    nc.sync.dma_start(tok[:], zeros[:, :16])

    nc.compile()
    return nc


def _prep(bert_embedding, x_bert_offset, x_mask):
    st = x_bert_offset[..., 0].astype(np.int64)
    ed = x_bert_offset[..., 1].astype(np.int64)
    length = ed - st
    valid = (x_mask > 0) & (length > 0)
    scale = np.where(
        valid, 1.0 / np.maximum(length, 1).astype(np.float64), 0.0
    ).astype(np.float32)
    st_ext = np.concatenate([st, ed[:, -1:]], axis=1)  # [B, W+1]

    # word index of each position (-1 if uncovered)
    word_of = np.full((B, S), -1, dtype=np.int64)
    s_idx = np.arange(S)
    for b in range(B):
        j = np.searchsorted(st_ext[b], s_idx, side="right") - 1
        ok = (j >= 0) & (j < W)
        word_of[b] = np.where(ok, j, -1)

    pairs = _active_pairs(st, ed)
    kl, mw = pairs

    E = np.ascontiguousarray(bert_embedding, dtype=np.float16)
    in_maps = []
    for c in range(NCORES):
        av = np.zeros((128, RPC * KT * 2), dtype=np.float32)
        for r in range(RPC):
            b = c * RPC + r
            for k in range(KT):
                if mw[r][k] is None:
                    continue
                j0 = mw[r][k][0] * 128
                col = (r * KT + k) * 2
                s = k * 128 + np.arange(128)
                wj = word_of[b, s]
                covered = wj >= 0
                # window hull guarantees covered words lie inside [j0, j0+wd)
                av[:, col] = np.where(covered, wj - j0, -1).astype(np.float32)
                av[:, col + 1] = np.where(
                    covered, scale[b, np.clip(wj, 0, W - 1)], 0.0
                )
        in_maps.append(
            {
                "E_in": E[c * RPC : (c + 1) * RPC],
                "av_in": av,
            }
        )
    return pairs, in_maps


def kernel(bert_embedding, x_bert_offset, x_mask):
    from concourse.bass_utils import run_bass_kernel_spmd

    bert_embedding = np.asarray(bert_embedding, dtype=np.float32)
    x_bert_offset = np.asarray(x_bert_offset)
    x_mask = np.asarray(x_mask)
    pairs, in_maps = _prep(bert_embedding, x_bert_offset, x_mask)
    key = repr(pairs)
    nc = _CACHE.get(key)
    if nc is None:
        nc = build_program(pairs)
        _CACHE[key] = nc
    res = run_bass_kernel_spmd(nc, in_maps, list(range(NCORES)))
    out = np.concatenate([res.results[c]["out"] for c in range(NCORES)], axis=0)
    return np.ascontiguousarray(out).astype(np.float32)



# revision 15
# speedup vs baseline: 51.8398x; 34.6630x over previous
"""Trainium2 Bass kernel for ragged subword mean pooling (nn_Bert).

Problem: out[b, j] = mean(bert_embedding[b, st_j:ed_j]) if (mask & ed>st) else 0
Shapes: bert_embedding [32, 1024, 768] f32, x_bert_offset [32, 768, 2] i32,
        x_mask [32, 768] i32 -> out [32, 768, 768] f32.

Strategy (pure data parallel, 4 batch rows per core on 8 cores):
  Spans are contiguous sorted segments, so per row the pooling is
  out = A.T @ E where A[s, j] = scale_j iff st_j <= s < ed_j
  (scale_j = valid/len folds the mean and mask directly into A).
  Each position s belongs to at most ONE word, so every A tile has at
  most one nonzero per partition row. The host ships just that
  (column, value) pair per position (~32KB/core) and the device
  reconstructs each [128, win] A window in a single fused DVE op
  against a constant column-index tile J:
      A[p, j] = (J[p, j] == idx_p) * val_p
  Only (m, k) tile pairs whose word/position ranges intersect are
  computed; the active-pair hull is derived on the host from the actual
  offsets (a superset is always correct since A is 0 outside).

All HBM I/O is fp16 (half the traffic of f32; this kernel is memory
bound). PE contracts fp16 at full rate into f32 PSUM. Metadata (word
indices <= 767, scales >= 1/1024) is fp16-exact; rel err ~2e-4.

Layouts are host-permuted so every DMA line is contiguous:
  E_in[r, p, k*D:+D] = E[r, k*128+p, :]   (12 KB/partition per row)
  out[r, p, m*D:+D]  = out[r, m*128+p, :] ( 9 KB/partition per row)

E loads are issued from the SP sequencer; each row's output store is
issued from the drain engine right after the row's last drain, so a
store waiting on compute never head-of-line-blocks the next row's E
load on SP (that stall serialized DMA behind compute, ~+15us).
"""

import sys

if "/opt/trn_rl_repo" not in sys.path:
    sys.path.insert(0, "/opt/trn_rl_repo")

import numpy as np

B, S, W, D = 32, 1024, 768, 768
NCORES = 8
RPC = B // NCORES  # rows per core
KT = S // 128  # 8 k-tiles (positions)
MT = W // 128  # 6 m-tiles (words)

_CACHE = {}


def _active_pairs(st, ed):
    """Per row-slot r: hull of active k-tiles for each m-tile, and hull of
    active m-tiles for each k-tile, unioned over cores (the SPMD program is
    shared by all 8 cores). A superset only costs time, never correctness.
    """
    kl = []
    for r in range(RPC):
        per_m = []
        for m in range(MT):
            klo, khi = KT, 0
            for c in range(NCORES):
                b = c * RPC + r
                s0 = int(st[b, m * 128 : (m + 1) * 128].min())
                s1 = int(ed[b, m * 128 : (m + 1) * 128].max())
                if s1 > s0:
                    klo = min(klo, s0 // 128)
                    khi = max(khi, (s1 + 127) // 128)
            per_m.append((klo, khi) if khi > klo else None)
        kl.append(per_m)

    mw = []
    for r in range(RPC):
        per_k = []
        for k in range(KT):
            mlo, mhi = MT, 0
            for m in range(MT):
                if kl[r][m] and kl[r][m][0] <= k < kl[r][m][1]:
                    mlo = min(mlo, m)
                    mhi = max(mhi, m + 1)
            per_k.append((mlo, mhi) if mhi > mlo else None)
        mw.append(per_k)
    return kl, mw


def build_program(pairs, repeat=1, drain="act", io="ext", stage=3, nodma=False,
                  ebufs=5, abufs=8, psbufs=3, obufs=3, avbufs=2):
    """Build the SPMD Bass program (one program, run on all 8 cores)."""
    import concourse.tile as tile
    from concourse import bacc, mybir

    kl, mw = pairs
    f32 = mybir.dt.float32
    f16 = mybir.dt.float16
    i32 = mybir.dt.int32
    AF = mybir.ActivationFunctionType
    OP = mybir.AluOpType

    nc = bacc.Bacc(
        "TRN2", target_bir_lowering=False, debug=False, num_devices=NCORES
    )

    E_in = nc.dram_tensor("E_in", [RPC, 128, KT * D], f16, kind="ExternalInput").ap()
    # packed per (r, k): column 2*(r*KT+k) = one-hot column index within the
    # A window (or -1), column +1 = A value (scale of the word at that
    # position, 0 if masked/empty/uncovered)
    av_in = nc.dram_tensor("av_in", [128, RPC * KT * 2], f32, kind="ExternalInput").ap()
    if io == "ext":
        out = nc.dram_tensor("out", [RPC, 128, MT * D], f16, kind="ExternalOutput").ap()
        tok = None
    else:
        out = nc.dram_tensor("out_scratch", [RPC, 128, MT * D], f16).ap()
        tok = nc.dram_tensor("tok", [128, 16], f16, kind="ExternalOutput").ap()
    outdma = not nodma

    def win(r, k):
        if mw[r][k] is None:
            return None
        mlo, mhi = mw[r][k]
        return mlo * 128, (mhi - mlo) * 128

    awidth = 128
    for r in range(RPC):
        for k in range(KT):
            if mw[r][k]:
                awidth = max(awidth, (mw[r][k][1] - mw[r][k][0]) * 128)

    drain_eng = None  # set inside context

    with tile.TileContext(nc) as tc:
        with (
            tc.tile_pool(name="const", bufs=1) as cpool,
            tc.tile_pool(name="E", bufs=ebufs) as epool,
            tc.tile_pool(name="bc", bufs=avbufs) as bcpool,
            tc.tile_pool(name="A", bufs=abufs) as apool,
            tc.tile_pool(name="outsb", bufs=obufs) as opool,
            tc.tile_pool(name="psum", bufs=psbufs, space="PSUM") as pspool,
        ):
            drain_eng = nc.scalar if drain == "act" else nc.vector

            # constant column-index tile J[p, j] = j
            j_i = cpool.tile([128, awidth], i32)
            nc.gpsimd.iota(j_i[:], pattern=[[1, awidth]], base=0, channel_multiplier=0)
            j_f = cpool.tile([128, awidth], f16)
            nc.vector.tensor_copy(j_f[:], j_i[:])
            zeros = cpool.tile([128, D], f16)
            nc.vector.memset(zeros[:], 0.0)
            econst = None
            if nodma:
                econst = cpool.tile([128, KT * D], f16, tag="Ec")
                nc.vector.memset(econst[:], 0.5)

            def drain_to(oslice, src):
                if drain == "act":
                    nc.scalar.activation(oslice, src, AF.Copy)
                else:
                    nc.vector.tensor_copy(oslice, src)

            last_at = None
            for _ in range(repeat):
                if not nodma:
                    av = bcpool.tile([128, RPC * KT * 2], f32, tag="av")
                    nc.sync.dma_start(av[:], av_in[:, :])

                for r in range(RPC):
                    # whole E row in one contiguous DMA (12 KB per partition)
                    if nodma:
                        erow = econst
                    else:
                        erow = epool.tile([128, KT * D], f16, tag="E")
                        nc.sync.dma_start(erow[:], E_in[r])
                    et = [erow[:, k * D : (k + 1) * D] for k in range(KT)]

                    # one-hot A windows, one fused DVE op per k-tile
                    ak = {}
                    for k in range(KT if (stage >= 1 and not nodma) else 0):
                        w = win(r, k)
                        if w is None:
                            continue
                        j0, wd = w
                        c = (r * KT + k) * 2
                        at = apool.tile([128, awidth], f16, tag="A")
                        nc.vector.tensor_scalar(
                            at[:, :wd],
                            j_f[:, :wd],
                            av[:, c : c + 1],
                            av[:, c + 1 : c + 2],
                            OP.is_equal,
                            OP.mult,
                        )
                        ak[k] = (at, j0)
                        last_at = at

                    otile = opool.tile([128, MT * D], f16, tag="osb")
                    for m in range(MT):
                        oslice = otile[:, m * D : (m + 1) * D]
                        if kl[r][m] is None or stage < 2 or not ak:
                            drain_to(oslice, zeros[:])
                            continue
                        klo, khi = kl[r][m]
                        ps = pspool.tile([128, D], f32, tag="ps")
                        for k in range(klo, khi):
                            at, j0 = ak[k]
                            lhsT = at[:, m * 128 - j0 : (m + 1) * 128 - j0]
                            first = k == klo
                            last = k == khi - 1
                            for n0 in range(0, D, 512):
                                n1 = min(n0 + 512, D)
                                nc.tensor.matmul(
                                    ps[:, n0:n1],
                                    lhsT,
                                    et[k][:, n0:n1],
                                    start=first,
                                    stop=last,
                                )
                        if stage < 3:
                            drain_to(oslice, zeros[:])
                            continue
                        drain_to(oslice, ps[:])
                    # store issued from the drain engine: no cross-engine
                    # wait, and it never blocks the SP load queue
                    if outdma:
                        drain_eng.dma_start(out[r], otile[:])

            if tok is not None:
                if last_at is not None:
                    nc.sync.dma_start(tok[:], last_at[:, :16])
                else:
                    nc.sync.dma_start(tok[:], zeros[:, :16])

    nc.compile()
    return nc


def _prep(bert_embedding, x_bert_offset, x_mask):
    st = x_bert_offset[..., 0].astype(np.int64)
    ed = x_bert_offset[..., 1].astype(np.int64)
    length = ed - st
    valid = (x_mask > 0) & (length > 0)
    scale = np.where(
        valid, 1.0 / np.maximum(length, 1).astype(np.float64), 0.0
    ).astype(np.float32)
    st_ext = np.concatenate([st, ed[:, -1:]], axis=1)  # [B, W+1]

    # word index of each position (-1 if uncovered)
    word_of = np.full((B, S), -1, dtype=np.int64)
    s_idx = np.arange(S)
    for b in range(B):
        j = np.searchsorted(st_ext[b], s_idx, side="right") - 1
        ok = (j >= 0) & (j < W)
        word_of[b] = np.where(ok, j, -1)

    pairs = _active_pairs(st, ed)
    kl, mw = pairs

    # permuted fp16 E: E_perm[b, p, k*D:+D] = E[b, k*128+p, :]
    E = np.ascontiguousarray(
        np.asarray(bert_embedding, dtype=np.float16)
        .reshape(B, KT, 128, D)
        .transpose(0, 2, 1, 3)
        .reshape(B, 128, KT * D)
    )
    in_maps = []
    for c in range(NCORES):
        av = np.zeros((128, RPC * KT * 2), dtype=np.float32)
        for r in range(RPC):
            b = c * RPC + r
            for k in range(KT):
                if mw[r][k] is None:
                    continue
                j0 = mw[r][k][0] * 128
                col = (r * KT + k) * 2
                s = k * 128 + np.arange(128)
                wj = word_of[b, s]
                covered = wj >= 0
                # window hull guarantees covered words lie inside [j0, j0+wd)
                av[:, col] = np.where(covered, wj - j0, -1).astype(np.float32)
                av[:, col + 1] = np.where(
                    covered, scale[b, np.clip(wj, 0, W - 1)], 0.0
                )
        in_maps.append(
            {
                "E_in": E[c * RPC : (c + 1) * RPC],
                "av_in": av,
            }
        )
    return pairs, in_maps


def kernel(bert_embedding, x_bert_offset, x_mask):
    from concourse.bass_utils import run_bass_kernel_spmd

    bert_embedding = np.asarray(bert_embedding, dtype=np.float32)
    x_bert_offset = np.asarray(x_bert_offset)
    x_mask = np.asarray(x_mask)
    pairs, in_maps = _prep(bert_embedding, x_bert_offset, x_mask)
    key = repr(pairs)
    nc = _CACHE.get(key)
    if nc is None:
        nc = build_program(pairs)
        _CACHE[key] = nc
    res = run_bass_kernel_spmd(nc, in_maps, list(range(NCORES)))
    # un-permute: out_dev[r, p, m*D:+D] = out[r, m*128+p, :]
    out = np.concatenate([res.results[c]["out"] for c in range(NCORES)], axis=0)
    out = (
        out.reshape(B, 128, MT, D)
        .transpose(0, 2, 1, 3)
        .reshape(B, W, D)
        .astype(np.float32)
    )
    return out
